# revision 34
# baseline (speedup 1.0000x reference)
"""DeepGO2 (MLP + GATConv + GO-embedding head) on 8 Trainium2 cores.

Sharding: the input MLP matmul is K-sharded (each core holds a 320-row
chunk of W1 and the matching feature columns for ALL nodes; a bf16
ReduceScatter sums partials and hands each core the xT shard for its
1250 nodes, padded to 1280); the GAT phase is data-parallel over nodes;
the logits head is tensor-parallel over GO columns (each core owns 1280
of the 10240 padded GO entries). Two AllGathers link the phases: a
per-node bf16/fp8 "payload" table (h | el | 1) feeds the edge
aggregation, and the aggregated xgT feeds the logits matmul. This keeps
every replicated upload small (~102MB total relay input vs 700MB+ for
naive replication).

Math identities used (all host-precomputable):
  el = (x@fc_w)@attn_l = x@(fc_w@attn_l)        (and er likewise)
  logits[n,g] = sigmoid(agg_n[n]@go[g] + s[n] + rad'[g])
    s[n]    = agg_n[n]@hasFunc  (computed as an extra goT column)
    rad'[g] = |go_rad[g]| + gat_bias@go[g] + gat_bias@hasFunc
  edge softmax needs no max-subtraction: |e| <= ~2 for this data regime,
  exp() is computed unshifted and normalized by z = sum_e w_e.

Output is produced as uint8 ([all 10240 padded nodes, core's 1280 GO
cols] per core; sigmoid quantized as trunc(255*p + 0.5), max abs error
~0.002 vs the 2e-2 gate) to quarter the relay download+zero-upload
traffic; the host rescales to f32 with 1/255.
"""

import os
import sys

for _p in ("/opt/trn_rl_repo", "/root/.axon_site/_ro/trn_rl_repo"):
    if os.path.isdir(_p) and _p not in sys.path:
        sys.path.insert(0, _p)

import numpy as np
import ml_dtypes

# ---------------------------------------------------------------- constants
N, E, IN, H, G, NZ, R = 10000, 320000, 2560, 1024, 10000, 5000, 10
NC = 8            # cores
NPC = 1250        # real nodes per core
NT = 10           # node tiles per core
NPCP = NT * 128   # padded nodes per core (1280)
IN_T = IN // 128  # 20
H_T = H // 128    # 8
PAY = 1280        # payload row BYTES: h fp8 (1024B) | el bf16 | 1.0 bf16 | pad
                  # (dma_gather requires the row size to be a multiple of 256)
W2C = H + 2       # fc_w | al2 | ar2
GP = 10240        # padded GO count (8 * GPC)
GPC = GP // NC    # GO columns per core (1280)
GOC = GPC + 1     # + hasFunc column (computes per-node s term)
CB = 4            # blocks per dma_gather chunk (512 edges)
BF16 = ml_dtypes.bfloat16


# ---------------------------------------------------------------- host prep
def _host_prep(inputs):
    f32 = np.float32
    features = np.asarray(inputs["features"], f32)
    src = np.asarray(inputs["src"]).astype(np.int64)
    dst = np.asarray(inputs["dst"]).astype(np.int64)
    W1 = np.asarray(inputs["W1"], f32)
    b1 = np.asarray(inputs["b1"], f32)
    fc_w = np.asarray(inputs["fc_w"], f32)
    attn_l = np.asarray(inputs["attn_l"], f32)
    attn_r = np.asarray(inputs["attn_r"], f32)
    gat_bias = np.asarray(inputs["gat_bias"], f32)
    go_embed = np.asarray(inputs["go_embed"], f32)
    go_rad = np.asarray(inputs["go_rad"], f32)
    rel_embed = np.asarray(inputs["rel_embed"], f32)

    hf = rel_embed[R]                      # hasFunc row  [H]
    al2 = fc_w @ attn_l                    # [H]
    ar2 = fc_w @ attn_r
    w2e = np.concatenate([fc_w, al2[:, None], ar2[:, None]], axis=1)

    go = go_embed[:G]                      # [G, H]
    goT = np.zeros((H, GP), f32)
    goT[:, :G] = go.T
    radp = np.zeros((1, GP), f32)
    radp[0, :G] = np.abs(go_rad[:G, 0]) + go @ gat_bias + float(gat_bias @ hf)

    b1p = b1.reshape(H_T, 128).T.copy()    # [128, H_T]

    # ---- edges: sort by (core, dst-tile), pad per tile ----
    dstc = dst // NPC
    dloc = dst % NPC
    tl = dloc // 128
    dcol = dloc % 128
    group = dstc * NT + tl                 # [E] in [0, 80)
    order = np.argsort(group, kind="stable")
    g_s = group[order]
    src_s = src[order]
    dcol_s = dcol[order]

    counts = np.bincount(group, minlength=NC * NT).reshape(NC, NT)
    maxcnt = counts.max(axis=0)            # per-tile max over cores
    nblk_t = [max(CB, ((int(m) + 127) // 128 + CB - 1) // CB * CB) for m in maxcnt]
    NBT = int(sum(nblk_t))
    EPC = NBT * 128
    blk_base = np.zeros(NT + 1, np.int64)
    blk_base[1:] = np.cumsum(nblk_t)

    # rank of each sorted edge within its group
    gstart = np.zeros(NC * NT + 1, np.int64)
    gstart[1:] = np.cumsum(np.bincount(group, minlength=NC * NT))
    rank = np.arange(E, dtype=np.int64) - gstart[g_s]

    core_s = g_s // NT
    tile_s = g_s % NT
    slot = blk_base[tile_s] * 128 + rank   # slot within the core's padded edges
    srow = NPCP * (src_s // NPC) + (src_s % NPC)  # padded payload row of src

    gi = np.zeros((NC, EPC), np.int16)
    gi[core_s, slot] = srow.astype(np.int16)
    # per-slot local dst column (-1 for padding slots)
    dstloc = np.full((NC, NBT, 128), -1.0, np.float32)
    dstloc[core_s, slot // 128, slot % 128] = dcol_s
    # [NC,128,NBT] bf16 (values in {-1, 0..127} are exact in bf16)
    dstloc = np.ascontiguousarray(dstloc.transpose(0, 2, 1)).astype(BF16)

    # wrap gather indices: idx i -> [i % 16, i // 16], replicated to 128 rows
    gi_w = np.ascontiguousarray(
        np.tile(gi.reshape(NC, EPC // 16, 16).transpose(0, 2, 1), (1, 8, 1))
    )                                       # [NC, 128, EPC//16] int16

    goT_b = goT.astype(BF16)
    hf_b = hf.astype(BF16)
    w2e_b = w2e.astype(BF16)

    # K-sharded first matmul: core c holds feature-columns chunk c (for ALL
    # nodes, padded per-block) and the matching W1 row chunk; partial xT is
    # summed across cores by an on-device ReduceScatter.
    INC = IN // NC  # 320 feature rows per core
    featc_all = np.zeros((IN, NC * NPCP), BF16)
    for b in range(NC):
        featc_all[:, b * NPCP : b * NPCP + NPC] = (
            features[b * NPC : (b + 1) * NPC].T.astype(BF16)
        )

    in_maps = []
    for c in range(NC):
        goc = np.empty((H, GOC), BF16)
        goc[:, :GPC] = goT_b[:, c * GPC : (c + 1) * GPC]
        goc[:, GPC] = hf_b
        in_maps.append(
            {
                "featc": np.ascontiguousarray(featc_all[c * INC : (c + 1) * INC]),
                "w1c": W1[c * INC : (c + 1) * INC].astype(BF16),
                "w2e": w2e_b,
                "b1p": b1p,
                "goc": goc,
                "radc": radp[:, c * GPC : (c + 1) * GPC].copy(),
                "gidx": gi_w[c],
                "dstloc": dstloc[c],
            }
        )
    return in_maps, nblk_t


# ---------------------------------------------------------------- device code
def build_nc(nblk_t, do_ag=True, do_b=True, do_c=True):
    import concourse.bacc as bacc
    import concourse.mybir as mybir
    import concourse.tile as tile
    from concourse import library_config
    from concourse.masks import make_identity
    from concourse.tile_autobufs import add_dep_helper

    dt = mybir.dt
    AF = mybir.ActivationFunctionType
    ALU = mybir.AluOpType

    NBT = int(sum(nblk_t))
    EPC = NBT * 128
    blk_base = [0]
    for nb in nblk_t:
        blk_base.append(blk_base[-1] + nb)

    nc = bacc.Bacc("TRN2", target_bir_lowering=False, debug=False, num_devices=NC)

    INC = IN // NC
    featc = nc.dram_tensor(
        "featc", [INC, NC * NPCP], dt.bfloat16, kind="ExternalInput"
    )
    w1c = nc.dram_tensor("w1c", [INC, H], dt.bfloat16, kind="ExternalInput")
    w2e = nc.dram_tensor("w2e", [H, W2C], dt.bfloat16, kind="ExternalInput")
    b1p = nc.dram_tensor("b1p", [128, H_T], dt.float32, kind="ExternalInput")
    goc = nc.dram_tensor("goc", [H, GOC], dt.bfloat16, kind="ExternalInput")
    radc = nc.dram_tensor("radc", [1, GPC], dt.float32, kind="ExternalInput")
    gidx = nc.dram_tensor("gidx", [128, EPC // 16], dt.int16, kind="ExternalInput")
    dstloc = nc.dram_tensor("dstloc", [128, NBT], dt.bfloat16, kind="ExternalInput")
    out = nc.dram_tensor("out", [NC * NPC, GPC], dt.uint8, kind="ExternalOutput")

    pay_local = nc.dram_tensor("pay_local", [NPCP, PAY], dt.uint8)
    pay_full = nc.dram_tensor(
        "pay_full", [NC * NPCP, PAY], dt.uint8, addr_space="Shared"
    )
    rs_in = nc.dram_tensor("rs_in", [NC * H, NPCP], dt.bfloat16)
    rs_out = nc.dram_tensor("rs_out", [H, NPCP], dt.bfloat16)
    xgT_local = nc.dram_tensor("xgT_local", [H, NPCP], dt.bfloat16)
    xgT_full = nc.dram_tensor(
        "xgT_full", [NC * H, NPCP], dt.bfloat16, addr_space="Shared"
    )

    with tile.TileContext(nc) as tc:
        lib_inst = nc.gpsimd.load_library(library_config.mlp)

        with (
            tc.tile_pool(name="const", bufs=1) as cp,
            tc.tile_pool(name="paydma", bufs=3) as paypool,
        ):
            ident = cp.tile([128, 128], dt.bfloat16)
            make_identity(nc, ident[:])
            ones1 = cp.tile([1, 128], dt.float32)
            nc.vector.memset(ones1[:], 1.0)
            ones1_bf = cp.tile([1, 128], dt.bfloat16)
            nc.vector.memset(ones1_bf[:], 1.0)
            iota_i = cp.tile([128, 128], dt.int32)
            nc.gpsimd.iota(iota_i[:], pattern=[[1, 128]], base=0, channel_multiplier=0)
            iota_bf = cp.tile([128, 128], dt.bfloat16)
            nc.vector.tensor_copy(iota_bf[:], iota_i[:])
            b1_sb = cp.tile([128, H_T], dt.float32)
            nc.sync.dma_start(b1_sb[:], b1p[:])
            er_sb = cp.tile([128, NT], dt.float32)
            er_bf = cp.tile([128, NT], dt.bfloat16)
            xg_sb = cp.tile([128, NT * H], dt.bfloat16)

            pay_dmas = []

            # ---------------- phase A0: partial xT for ALL nodes ----------
            # Each core contracts its 320-row K-chunk of W1 against its
            # feature-column chunk for all 10240 padded nodes; a bf16
            # ReduceScatter sums the 8 partials and hands each core the xT
            # shard for its own 1280 nodes.
            KT = [(0, 128), (128, 128), (256, INC - 256)]  # k-tile (row0, rows)
            with tc.tile_pool(name="phA", bufs=1) as ap:
                w1_sb = ap.tile([128, len(KT), H], dt.bfloat16)
                nc.sync.dma_start(
                    w1_sb[:, 0:2, :],
                    w1c.ap()[0:256, :].rearrange("(k p) j -> p k j", p=128),
                )
                nc.sync.dma_start(w1_sb[0 : KT[2][1], 2, :], w1c.ap()[256:INC, :])
                ft_sb = ap.tile([128, len(KT), NC * NPCP], dt.bfloat16)
                nc.sync.dma_start(
                    ft_sb[:, 0:2, :],
                    featc.ap()[0:256, :].rearrange("(k p) n -> p k n", p=128),
                )
                nc.sync.dma_start(
                    ft_sb[0 : KT[2][1], 2, :], featc.ap()[256:INC, :]
                )
                w2_sb = ap.tile([128, H_T, W2C], dt.bfloat16)
                nc.sync.dma_start(
                    w2_sb[:], w2e.ap().rearrange("(k p) j -> p k j", p=128)
                )
                xT_sb = ap.tile([128, H_T * NPCP], dt.bfloat16)

                rs_dmas = []
                with (
                    tc.tile_pool(name="psX", bufs=3, space="PSUM") as psx,
                    tc.tile_pool(name="rsst", bufs=6) as rsp,
                ):
                    for j in range(H_T):
                        for d in range(NC):
                            for co, cw in [(0, 512), (512, 512), (1024, 256)]:
                                ps = psx.tile(
                                    [128, cw], dt.float32, tag=f"psx{cw}"
                                )
                                for k, (k0, kw) in enumerate(KT):
                                    nc.tensor.matmul(
                                        ps[:],
                                        w1_sb[0:kw, k, j * 128 : (j + 1) * 128],
                                        ft_sb[0:kw, k, d * NPCP + co :
                                              d * NPCP + co + cw],
                                        start=(k == 0),
                                        stop=(k == len(KT) - 1),
                                    )
                                sb = rsp.tile([128, cw], dt.bfloat16, tag=f"rs{cw}")
                                nc.vector.tensor_copy(sb[:], ps[:])
                                rd = nc.sync.dma_start(
                                    rs_in[
                                        d * H + j * 128 : d * H + (j + 1) * 128,
                                        co : co + cw,
                                    ],
                                    sb[:],
                                )
                                rs_dmas.append(rd)

                rs = nc.gpsimd.collective_compute(
                    "ReduceScatter",
                    ALU.add,
                    replica_groups=[list(range(NC))],
                    ins=[rs_in[:]],
                    outs=[rs_out[:]],
                )
                for rd in rs_dmas:
                    add_dep_helper(rs.ins, rd.ins, sync=True, reason="rs after part")

                rsld = ap.tile([128, H_T, NPCP], dt.bfloat16)
                rld = nc.sync.dma_start(
                    rsld[:], rs_out.ap().rearrange("(k p) n -> p k n", p=128)
                )
                add_dep_helper(rld.ins, rs.ins, sync=True, reason="load after rs")
                for k in range(H_T):
                    nc.scalar.activation(
                        xT_sb[:, k * NPCP : (k + 1) * NPCP],
                        rsld[:, k, :],
                        AF.Relu,
                        bias=b1_sb[:, k : k + 1],
                    )

                with (
                    tc.tile_pool(name="psH", bufs=3, space="PSUM") as psh_p,
                    tc.tile_pool(name="psS", bufs=2, space="PSUM") as pss_p,
                ):
                  for n in range(NT):
                    psh = psh_p.tile([128, H], dt.float32)
                    pss = pss_p.tile([128, 2], dt.float32)
                    for fo in range(0, H, 512):
                        for k in range(H_T):
                            nc.tensor.matmul(
                                psh[:, fo : fo + 512],
                                xT_sb[:, k * NPCP + n * 128 : k * NPCP + (n + 1) * 128],
                                w2_sb[:, k, fo : fo + 512],
                                start=(k == 0),
                                stop=(k == H_T - 1),
                            )
                    for k in range(H_T):
                        nc.tensor.matmul(
                            pss[:],
                            xT_sb[:, k * NPCP + n * 128 : k * NPCP + (n + 1) * 128],
                            w2_sb[:, k, H : H + 2],
                            start=(k == 0),
                            stop=(k == H_T - 1),
                        )
                    pay = paypool.tile([128, PAY], dt.uint8)
                    nc.vector.tensor_copy(
                        pay[:, 0:H].bitcast(dt.float8e4), psh[:]
                    )
                    side = pay[:, H:PAY].bitcast(dt.bfloat16)
                    nc.vector.tensor_copy(side[:, 0:1], pss[:, 0:1])
                    nc.vector.memset(side[:, 1:2], 1.0)
                    nc.vector.memset(side[:, 2:128], 0.0)
                    nc.vector.tensor_copy(er_sb[:, n : n + 1], pss[:, 1:2])
                    d = nc.sync.dma_start(
                        pay_local[n * 128 : (n + 1) * 128, :], pay[:]
                    )
                    pay_dmas.append(d)
                nc.vector.tensor_copy(er_bf[:], er_sb[:])

            # ---------------- AllGather payload ---------------------------
            if not do_ag:
                do_b = False
            cc = None
            if do_ag:
              cc = nc.gpsimd.collective_compute(
                "AllGather",
                ALU.bypass,
                replica_groups=[list(range(NC))],
                ins=[pay_local[:]],
                outs=[pay_full[:]],
              )
            if cc is not None:
              for d in pay_dmas:
                add_dep_helper(cc.ins, d.ins, sync=True, reason="cc after payload")

            # ---------------- phase B: edge aggregation -------------------
            if do_b:
              with (
                tc.tile_pool(name="phB", bufs=1) as bp,
                tc.tile_pool(name="erbc", bufs=2) as ebp,
                tc.tile_pool(name="gat", bufs=5) as gp,
                tc.tile_pool(name="lw", bufs=4) as lwp,
                tc.tile_pool(name="psAgg", bufs=1, space="PSUM") as psagg,
                tc.tile_pool(name="psEr", bufs=2, space="PSUM") as pser,
                tc.tile_pool(name="small", bufs=4) as smp,
            ):
                gidx_sb = bp.tile([128, EPC // 16], dt.int16)
                nc.sync.dma_start(gidx_sb[:], gidx[:])
                dl_sb = bp.tile([128, NBT], dt.bfloat16)
                nc.sync.dma_start(dl_sb[:], dstloc[:])

                for t in range(NT):
                    nbt = nblk_t[t]
                    # er_bc[e, d] = er[tile t][d]  — 2-matmul partition broadcast
                    erp1 = pser.tile([1, 128], dt.float32, tag="erp1")
                    nc.tensor.matmul(erp1[:], er_bf[:, t : t + 1], ident[:])
                    erow = smp.tile([1, 128], dt.bfloat16, tag="erow")
                    nc.vector.tensor_copy(erow[:], erp1[:])
                    erp2 = pser.tile([128, 128], dt.float32, tag="erp2")
                    nc.tensor.matmul(erp2[:], ones1_bf[:], erow[:])
                    er_bc = ebp.tile([128, 128], dt.bfloat16, tag="erbc")
                    nc.vector.tensor_copy(er_bc[:], erp2[:])

                    ps0 = psagg.tile([128, 512], dt.float32, tag="agg0")
                    ps1 = psagg.tile([128, 512], dt.float32, tag="agg1")
                    psz = psagg.tile([128, 1], dt.float32, tag="aggz")

                    for c in range(nbt // CB):
                        gt = gp.tile([128, CB, PAY], dt.uint8, tag="gat")
                        icol = (blk_base[t] + c * CB) * 8
                        gd = nc.gpsimd.dma_gather(
                            gt[:],
                            pay_full[:],
                            gidx_sb[:, icol : icol + CB * 8],
                            CB * 128,
                            CB * 128,
                            PAY,
                        )
                        add_dep_helper(gd.ins, lib_inst.ins, sync=False,
                                       reason="gather after lib")
                        add_dep_helper(gd.ins, cc.ins, sync=True,
                                       reason="gather after allgather")
                        blk0 = c * CB
                        # whole-chunk edge softmax weights (CB blocks at once,
                        # per-block values read through stride-0 broadcasts)
                        el_b = (
                            gt[:, :, H : H + 2]
                            .bitcast(dt.bfloat16)
                            .broadcast_to([128, CB, 128])
                        )
                        er_b = er_bc[:].rearrange(
                            "p (c d) -> p c d", c=1
                        ).broadcast_to([128, CB, 128])
                        es = lwp.tile([128, CB, 128], dt.bfloat16, tag="es")
                        nc.vector.tensor_tensor(es[:], er_b, el_b, op=ALU.add)
                        # lr = leaky_relu(es) = max(0.2*es, es)
                        lr = lwp.tile([128, CB, 128], dt.bfloat16, tag="lr")
                        nc.vector.scalar_tensor_tensor(
                            lr[:], es[:], 0.2, es[:], op0=ALU.mult, op1=ALU.max
                        )
                        # w = exp(lr)
                        wt = lwp.tile([128, CB, 128], dt.bfloat16, tag="wt")
                        nc.scalar.activation(wt[:], lr[:], AF.Exp)
                        # lw = (iota == dstloc) * w
                        iota_b = iota_bf[:].rearrange(
                            "p (c d) -> p c d", c=1
                        ).broadcast_to([128, CB, 128])
                        dl_b = (
                            dl_sb[:, blk_base[t] + blk0 : blk_base[t] + blk0 + CB]
                            .rearrange("p (c o) -> p c o", o=1)
                            .broadcast_to([128, CB, 128])
                        )
                        eq = lwp.tile([128, CB, 128], dt.bfloat16, tag="eq")
                        nc.vector.tensor_tensor(eq[:], iota_b, dl_b, op=ALU.is_equal)
                        lw = lwp.tile([128, CB, 128], dt.bfloat16, tag="lw")
                        nc.vector.tensor_tensor(lw[:], eq[:], wt[:], op=ALU.mult)
                        for b in range(CB):
                            blk = blk0 + b
                            first = blk == 0
                            last = blk == nbt - 1
                            h8 = gt[:, b, 0:H].bitcast(dt.float8e4)
                            one_b = gt[:, b, H + 2 : H + 4].bitcast(dt.bfloat16)
                            nc.tensor.matmul(
                                ps0[:], lw[:, b, :], h8[:, 0:512],
                                start=first, stop=last,
                            )
                            nc.tensor.matmul(
                                ps1[:], lw[:, b, :], h8[:, 512:1024],
                                start=first, stop=last,
                            )
                            nc.tensor.matmul(
                                psz[:], lw[:, b, :], one_b[:],
                                start=first, stop=last,
                            )

                    zc = smp.tile([128, 1], dt.float32, tag="zc")
                    nc.vector.tensor_scalar_max(zc[:], psz[:], 1e-30)
                    rz = smp.tile([128, 1], dt.float32, tag="rz")
                    nc.vector.reciprocal(rz[:], zc[:])
                    nc.scalar.mul(xg_sb[:, t * H : t * H + 512], ps0[:], rz[:])
                    nc.scalar.mul(xg_sb[:, t * H + 512 : (t + 1) * H], ps1[:], rz[:])

            # ---------------- transpose xg + AllGather xgT ----------------
            cc2 = None
            if do_b:
                with tc.tile_pool(name="phT", bufs=1) as tp:
                    xgT_sb = tp.tile([128, H_T, NPCP], dt.bfloat16)
                    with tc.tile_pool(name="psT", bufs=4, space="PSUM") as pst_p:
                        for t in range(NT):
                            for k in range(H_T):
                                pst = pst_p.tile([128, 128], dt.bfloat16, tag="pst")
                                nc.tensor.transpose(
                                    pst[:],
                                    xg_sb[:, t * H + k * 128 : t * H + (k + 1) * 128],
                                    ident[:],
                                )
                                nc.vector.tensor_copy(
                                    xgT_sb[:, k, t * 128 : (t + 1) * 128],
                                    pst[:],
                                )
                    xd = nc.sync.dma_start(
                        xgT_local.ap().rearrange("(k p) n -> p k n", p=128),
                        xgT_sb[:],
                    )
                cc2 = nc.gpsimd.collective_compute(
                    "AllGather",
                    ALU.bypass,
                    replica_groups=[list(range(NC))],
                    ins=[xgT_local[:]],
                    outs=[xgT_full[:]],
                )
                add_dep_helper(cc2.ins, xd.ins, sync=True, reason="cc2 after xgT")

            # ---------------- phase C: logits ----------------------------
            if not do_c:
                dum = paypool.tile([128, 512], dt.uint8, tag="dum")
                nc.vector.memset(dum[:], 0)
                nc.sync.dma_start(out[0:128, 0:512], dum[:])
            if do_c:
              GB = [(0, 512), (512, 512), (1024, 257)]  # (col offset, psum width)
              with (
                tc.tile_pool(name="phC", bufs=1) as cpc,
                tc.tile_pool(name="xtp", bufs=3) as xtp,
                tc.tile_pool(name="outp", bufs=6) as outp,
                tc.tile_pool(name="sfp", bufs=3) as sfp,
                tc.tile_pool(name="psC", bufs=2, space="PSUM") as psc_p,
              ):
                go_sb = cpc.tile([128, H_T, GOC], dt.bfloat16)
                nc.sync.dma_start(
                    go_sb[:], goc.ap().rearrange("(k p) g -> p k g", p=128)
                )
                rad_sb = cpc.tile([1, GPC], dt.float32)
                nc.sync.dma_start(rad_sb[:], radc[:])
                rad_bc = cpc.tile([128, GPC], dt.bfloat16)
                for g0, w in GB:
                    wr = min(w, GPC - g0)
                    psr = psc_p.tile([128, wr], dt.float32, tag="psrad")
                    nc.tensor.matmul(psr[:], ones1[:], rad_sb[:, g0 : g0 + wr])
                    nc.vector.tensor_copy(rad_bc[:, g0 : g0 + wr], psr[:])

                for m in range(NC * NT):
                    bk, tl = divmod(m, NT)
                    row0 = bk * NPC + tl * 128
                    nrows = min(128, NPC - tl * 128)
                    xt = xtp.tile([128, H_T, 128], dt.bfloat16, tag="xt")
                    xld = nc.sync.dma_start(
                        xt[:],
                        xgT_full.ap()[
                            bk * H : (bk + 1) * H, tl * 128 : (tl + 1) * 128
                        ].rearrange("(k p) n -> p k n", p=128),
                    )
                    add_dep_helper(xld.ins, cc2.ins, sync=True,
                                   reason="xt after xgT allgather")
                    pss = [
                        psc_p.tile(
                            [128, w], dt.float32, tag=f"psc{i}", name=f"psc{i}"
                        )
                        for i, (g0, w) in enumerate(GB)
                    ]
                    for k in range(H_T):
                        for i, (g0, w) in enumerate(GB):
                            nc.tensor.matmul(
                                pss[i][:],
                                xt[:, k, :],
                                go_sb[:, k, g0 : g0 + w],
                                start=(k == 0),
                                stop=(k == H_T - 1),
                            )
                    sf = sfp.tile([128, 1], dt.float32, tag="sf")
                    nc.vector.tensor_copy(sf[:], pss[2][:, 256:257])
                    for i, (g0, w) in enumerate(GB):
                        wo = min(w, GPC - g0)
                        st = outp.tile([128, wo], dt.bfloat16, tag=f"st{i}")
                        nc.vector.scalar_tensor_tensor(
                            st[:],
                            pss[i][:, 0:wo],
                            sf[:],
                            rad_bc[:, g0 : g0 + wo],
                            op0=ALU.add,
                            op1=ALU.add,
                        )
                        ot = outp.tile([128, wo], dt.float32, tag=f"ot{i}")
                        nc.scalar.activation(ot[:], st[:], AF.Sigmoid)
                        # quantize to uint8: trunc(255*sigmoid + 0.5)
                        ou = outp.tile([128, wo], dt.uint8, tag=f"ou{i}")
                        nc.vector.tensor_scalar(
                            ou[:], ot[:], 255.0, 0.5, op0=ALU.mult, op1=ALU.add
                        )
                        nc.sync.dma_start(
                            out[row0 : row0 + nrows, g0 : g0 + wo],
                            ou[0:nrows, :],
                        )

    nc.compile()
    return nc


# ---------------------------------------------------------------- entry point
def _assemble(results):
    full = np.empty((N, G), np.float32)
    for c in range(NC):
        g0 = c * GPC
        gw = min(GPC, G - g0)
        r = np.asarray(results[c]["out"])  # [N, GPC] uint8, rows in global order
        np.multiply(
            r[:, :gw],
            np.float32(1.0 / 255.0),
            out=full[:, g0 : g0 + gw],
        )
    return full


def kernel(**inputs):
    from concourse.bass_utils import run_bass_kernel_spmd

    in_maps, nblk_t = _host_prep(inputs)
    nc = build_nc(nblk_t)
    res = run_bass_kernel_spmd(nc, in_maps, list(range(NC)))
    return _assemble(res.results)


if __name__ == "__main__":
    # quick self-run with random data (no reference check)
    rng = np.random.default_rng(0)
    ins = {
        "features": rng.standard_normal((N, IN)).astype(np.float32),
        "src": rng.integers(0, N, E),
        "dst": rng.integers(0, N, E),
        "W1": rng.standard_normal((IN, H)).astype(np.float32) * 0.02,
        "b1": np.zeros(H, np.float32),
        "fc_w": rng.standard_normal((H, H)).astype(np.float32) * 0.02,
        "attn_l": rng.standard_normal(H).astype(np.float32) * 0.02,
        "attn_r": rng.standard_normal(H).astype(np.float32) * 0.02,
        "gat_bias": np.zeros(H, np.float32),
        "go_embed": rng.standard_normal((G + NZ, H)).astype(np.float32) * 0.02,
        "go_rad": rng.standard_normal((G + NZ, 1)).astype(np.float32) * 0.02,
        "rel_embed": rng.standard_normal((R + 1, H)).astype(np.float32) * 0.02,
    }
    out = kernel(**ins)
    print("out", out.shape, out.dtype, out[:2, :4])


# revision 40
# speedup vs baseline: 1.0215x; 1.0215x over previous
"""DeepGO2 (MLP + GATConv + GO-embedding head) on 8 Trainium2 cores.

Sharding: the input MLP matmul is K-sharded (each core holds a 320-row
chunk of W1 and the matching feature columns for ALL nodes; a bf16
ReduceScatter sums partials and hands each core the xT shard for its
1250 nodes, padded to 1280); the GAT phase is data-parallel over nodes;
the logits head is tensor-parallel over GO columns (each core owns 1280
of the 10240 padded GO entries). Two AllGathers link the phases: a
per-node bf16/fp8 "payload" table (h | el | 1) feeds the edge
aggregation, and the aggregated xgT feeds the logits matmul. This keeps
every replicated upload small (~102MB total relay input vs 700MB+ for
naive replication).

Math identities used (all host-precomputable):
  el = (x@fc_w)@attn_l = x@(fc_w@attn_l)        (and er likewise)
  logits[n,g] = sigmoid(agg_n[n]@go[g] + s[n] + rad'[g])
    s[n]    = agg_n[n]@hasFunc  (computed as an extra goT column)
    rad'[g] = |go_rad[g]| + gat_bias@go[g] + gat_bias@hasFunc
  edge softmax needs no max-subtraction: |e| <= ~2 for this data regime,
  exp() is computed unshifted and normalized by z = sum_e w_e.

Output is produced as uint8 ([all 10240 padded nodes, core's 1280 GO
cols] per core; sigmoid quantized as trunc(255*p + 0.5), max abs error
~0.002 vs the 2e-2 gate) to quarter the relay download+zero-upload
traffic; the host rescales to f32 with 1/255.
"""

import os
import sys

for _p in ("/opt/trn_rl_repo", "/root/.axon_site/_ro/trn_rl_repo"):
    if os.path.isdir(_p) and _p not in sys.path:
        sys.path.insert(0, _p)

import numpy as np
import ml_dtypes

# ---------------------------------------------------------------- constants
N, E, IN, H, G, NZ, R = 10000, 320000, 2560, 1024, 10000, 5000, 10
NC = 8            # cores
NPC = 1250        # real nodes per core
NT = 10           # node tiles per core
NPCP = NT * 128   # padded nodes per core (1280)
IN_T = IN // 128  # 20
H_T = H // 128    # 8
PAY = 1280        # payload row BYTES: h fp8 (1024B) | el bf16 | 1.0 bf16 | pad
                  # (dma_gather requires the row size to be a multiple of 256)
W2C = H + 2       # fc_w | al2 | ar2
GP = 10240        # padded GO count (8 * GPC)
GPC = GP // NC    # GO columns per core (1280)
GOC = GPC + 1     # + hasFunc column (computes per-node s term)
CB = 8            # blocks per dma_gather chunk (1024 edges)
BF16 = ml_dtypes.bfloat16


# ---------------------------------------------------------------- host prep
def _host_prep(inputs):
    f32 = np.float32
    features = np.asarray(inputs["features"], f32)
    src = np.asarray(inputs["src"]).astype(np.int64)
    dst = np.asarray(inputs["dst"]).astype(np.int64)
    W1 = np.asarray(inputs["W1"], f32)
    b1 = np.asarray(inputs["b1"], f32)
    fc_w = np.asarray(inputs["fc_w"], f32)
    attn_l = np.asarray(inputs["attn_l"], f32)
    attn_r = np.asarray(inputs["attn_r"], f32)
    gat_bias = np.asarray(inputs["gat_bias"], f32)
    go_embed = np.asarray(inputs["go_embed"], f32)
    go_rad = np.asarray(inputs["go_rad"], f32)
    rel_embed = np.asarray(inputs["rel_embed"], f32)

    hf = rel_embed[R]                      # hasFunc row  [H]
    al2 = fc_w @ attn_l                    # [H]
    ar2 = fc_w @ attn_r
    w2e = np.concatenate([fc_w, al2[:, None], ar2[:, None]], axis=1)

    go = go_embed[:G]                      # [G, H]
    goT = np.zeros((H, GP), f32)
    goT[:, :G] = go.T
    radp = np.zeros((1, GP), f32)
    radp[0, :G] = np.abs(go_rad[:G, 0]) + go @ gat_bias + float(gat_bias @ hf)

    b1p = b1.reshape(H_T, 128).T.copy()    # [128, H_T]

    # ---- edges: sort by (core, dst-tile), pad per tile ----
    dstc = dst // NPC
    dloc = dst % NPC
    tl = dloc // 128
    dcol = dloc % 128
    group = dstc * NT + tl                 # [E] in [0, 80)
    order = np.argsort(group, kind="stable")
    g_s = group[order]
    src_s = src[order]
    dcol_s = dcol[order]

    counts = np.bincount(group, minlength=NC * NT).reshape(NC, NT)
    maxcnt = counts.max(axis=0)            # per-tile max over cores
    nblk_t = [max(CB, ((int(m) + 127) // 128 + CB - 1) // CB * CB) for m in maxcnt]
    NBT = int(sum(nblk_t))
    EPC = NBT * 128
    blk_base = np.zeros(NT + 1, np.int64)
    blk_base[1:] = np.cumsum(nblk_t)

    # rank of each sorted edge within its group
    gstart = np.zeros(NC * NT + 1, np.int64)
    gstart[1:] = np.cumsum(np.bincount(group, minlength=NC * NT))
    rank = np.arange(E, dtype=np.int64) - gstart[g_s]

    core_s = g_s // NT
    tile_s = g_s % NT
    slot = blk_base[tile_s] * 128 + rank   # slot within the core's padded edges
    srow = NPCP * (src_s // NPC) + (src_s % NPC)  # padded payload row of src

    gi = np.zeros((NC, EPC), np.int16)
    gi[core_s, slot] = srow.astype(np.int16)
    # per-slot local dst column (-1 for padding slots)
    dstloc = np.full((NC, NBT, 128), -1.0, np.float32)
    dstloc[core_s, slot // 128, slot % 128] = dcol_s
    # [NC,128,NBT] bf16 (values in {-1, 0..127} are exact in bf16)
    dstloc = np.ascontiguousarray(dstloc.transpose(0, 2, 1)).astype(BF16)

    # wrap gather indices: idx i -> [i % 16, i // 16], replicated to 128 rows
    gi_w = np.ascontiguousarray(
        np.tile(gi.reshape(NC, EPC // 16, 16).transpose(0, 2, 1), (1, 8, 1))
    )                                       # [NC, 128, EPC//16] int16

    goT_b = goT.astype(BF16)
    hf_b = hf.astype(BF16)
    w2e_b = w2e.astype(BF16)

    # K-sharded first matmul: core c holds feature-columns chunk c (for ALL
    # nodes, padded per-block) and the matching W1 row chunk; partial xT is
    # summed across cores by an on-device ReduceScatter.
    INC = IN // NC  # 320 feature rows per core
    featc_all = np.zeros((IN, NC * NPCP), BF16)
    for b in range(NC):
        featc_all[:, b * NPCP : b * NPCP + NPC] = (
            features[b * NPC : (b + 1) * NPC].T.astype(BF16)
        )

    in_maps = []
    for c in range(NC):
        goc = np.empty((H, GOC), BF16)
        goc[:, :GPC] = goT_b[:, c * GPC : (c + 1) * GPC]
        goc[:, GPC] = hf_b
        in_maps.append(
            {
                "featc": np.ascontiguousarray(featc_all[c * INC : (c + 1) * INC]),
                "w1c": W1[c * INC : (c + 1) * INC].astype(BF16),
                "w2e": w2e_b,
                "b1p": b1p,
                "goc": goc,
                "radc": radp[:, c * GPC : (c + 1) * GPC].copy(),
                "gidx": gi_w[c],
                "dstloc": dstloc[c],
            }
        )
    return in_maps, nblk_t


# ---------------------------------------------------------------- device code
def build_nc(nblk_t, do_ag=True, do_b=True, do_c=True):
    import concourse.bacc as bacc
    import concourse.mybir as mybir
    import concourse.tile as tile
    from concourse import library_config
    from concourse.masks import make_identity
    from concourse.tile_autobufs import add_dep_helper

    dt = mybir.dt
    AF = mybir.ActivationFunctionType
    ALU = mybir.AluOpType

    NBT = int(sum(nblk_t))
    EPC = NBT * 128
    blk_base = [0]
    for nb in nblk_t:
        blk_base.append(blk_base[-1] + nb)

    nc = bacc.Bacc("TRN2", target_bir_lowering=False, debug=False, num_devices=NC)

    INC = IN // NC
    featc = nc.dram_tensor(
        "featc", [INC, NC * NPCP], dt.bfloat16, kind="ExternalInput"
    )
    w1c = nc.dram_tensor("w1c", [INC, H], dt.bfloat16, kind="ExternalInput")
    w2e = nc.dram_tensor("w2e", [H, W2C], dt.bfloat16, kind="ExternalInput")
    b1p = nc.dram_tensor("b1p", [128, H_T], dt.float32, kind="ExternalInput")
    goc = nc.dram_tensor("goc", [H, GOC], dt.bfloat16, kind="ExternalInput")
    radc = nc.dram_tensor("radc", [1, GPC], dt.float32, kind="ExternalInput")
    gidx = nc.dram_tensor("gidx", [128, EPC // 16], dt.int16, kind="ExternalInput")
    dstloc = nc.dram_tensor("dstloc", [128, NBT], dt.bfloat16, kind="ExternalInput")
    out = nc.dram_tensor("out", [NC * NPC, GPC], dt.uint8, kind="ExternalOutput")

    pay_local = nc.dram_tensor("pay_local", [NPCP, PAY], dt.uint8)
    pay_full = nc.dram_tensor(
        "pay_full", [NC * NPCP, PAY], dt.uint8, addr_space="Shared"
    )
    rs_in = nc.dram_tensor("rs_in", [NC * H, NPCP], dt.bfloat16)
    rs_out = nc.dram_tensor("rs_out", [H, NPCP], dt.bfloat16)
    xgT_local = nc.dram_tensor("xgT_local", [H, NPCP], dt.bfloat16)
    xgT_full = nc.dram_tensor(
        "xgT_full", [NC * H, NPCP], dt.bfloat16, addr_space="Shared"
    )

    with tile.TileContext(nc) as tc:
        lib_inst = nc.gpsimd.load_library(library_config.mlp)

        with (
            tc.tile_pool(name="const", bufs=1) as cp,
            tc.tile_pool(name="paydma", bufs=3) as paypool,
        ):
            ident = cp.tile([128, 128], dt.bfloat16)
            make_identity(nc, ident[:])
            ones1 = cp.tile([1, 128], dt.float32)
            nc.vector.memset(ones1[:], 1.0)
            ones1_bf = cp.tile([1, 128], dt.bfloat16)
            nc.vector.memset(ones1_bf[:], 1.0)
            iota_i = cp.tile([128, 128], dt.int32)
            nc.gpsimd.iota(iota_i[:], pattern=[[1, 128]], base=0, channel_multiplier=0)
            iota_bf = cp.tile([128, 128], dt.bfloat16)
            nc.vector.tensor_copy(iota_bf[:], iota_i[:])
            b1_sb = cp.tile([128, H_T], dt.float32)
            nc.sync.dma_start(b1_sb[:], b1p[:])
            er_sb = cp.tile([128, NT], dt.float32)
            er_bf = cp.tile([128, NT], dt.bfloat16)
            xg_sb = cp.tile([128, NT * H], dt.bfloat16)

            pay_dmas = []

            # ---------------- phase A0: partial xT for ALL nodes ----------
            # Each core contracts its 320-row K-chunk of W1 against its
            # feature-column chunk for all 10240 padded nodes; a bf16
            # ReduceScatter sums the 8 partials and hands each core the xT
            # shard for its own 1280 nodes.
            KT = [(0, 128), (128, 128), (256, INC - 256)]  # k-tile (row0, rows)
            with tc.tile_pool(name="phA", bufs=1) as ap:
                w1_sb = ap.tile([128, len(KT), H], dt.bfloat16)
                nc.sync.dma_start(
                    w1_sb[:, 0:2, :],
                    w1c.ap()[0:256, :].rearrange("(k p) j -> p k j", p=128),
                )
                nc.sync.dma_start(w1_sb[0 : KT[2][1], 2, :], w1c.ap()[256:INC, :])
                ft_sb = ap.tile([128, len(KT), NC * NPCP], dt.bfloat16)
                nc.sync.dma_start(
                    ft_sb[:, 0:2, :],
                    featc.ap()[0:256, :].rearrange("(k p) n -> p k n", p=128),
                )
                nc.sync.dma_start(
                    ft_sb[0 : KT[2][1], 2, :], featc.ap()[256:INC, :]
                )
                w2_sb = ap.tile([128, H_T, W2C], dt.bfloat16)
                nc.sync.dma_start(
                    w2_sb[:], w2e.ap().rearrange("(k p) j -> p k j", p=128)
                )
                xT_sb = ap.tile([128, H_T * NPCP], dt.bfloat16)

                rs_dmas = []
                with (
                    tc.tile_pool(name="psX", bufs=3, space="PSUM") as psx,
                    tc.tile_pool(name="rsst", bufs=4) as rsp,
                ):
                    for j in range(H_T):
                        for d in range(NC):
                            sb = rsp.tile([128, NPCP], dt.bfloat16, tag="rsst")
                            for co, cw in [(0, 512), (512, 512), (1024, 256)]:
                                ps = psx.tile(
                                    [128, cw], dt.float32, tag=f"psx{cw}"
                                )
                                for k, (k0, kw) in enumerate(KT):
                                    nc.tensor.matmul(
                                        ps[:],
                                        w1_sb[0:kw, k, j * 128 : (j + 1) * 128],
                                        ft_sb[0:kw, k, d * NPCP + co :
                                              d * NPCP + co + cw],
                                        start=(k == 0),
                                        stop=(k == len(KT) - 1),
                                    )
                                nc.vector.tensor_copy(
                                    sb[:, co : co + cw], ps[:]
                                )
                            rd = nc.sync.dma_start(
                                rs_in[d * H + j * 128 : d * H + (j + 1) * 128, :],
                                sb[:],
                            )
                            rs_dmas.append(rd)

                rs = nc.gpsimd.collective_compute(
                    "ReduceScatter",
                    ALU.add,
                    replica_groups=[list(range(NC))],
                    ins=[rs_in[:]],
                    outs=[rs_out[:]],
                )
                for rd in rs_dmas:
                    add_dep_helper(rs.ins, rd.ins, sync=True, reason="rs after part")

                rsld = ap.tile([128, H_T, NPCP], dt.bfloat16)
                rld = nc.sync.dma_start(
                    rsld[:], rs_out.ap().rearrange("(k p) n -> p k n", p=128)
                )
                add_dep_helper(rld.ins, rs.ins, sync=True, reason="load after rs")
                for k in range(H_T):
                    nc.scalar.activation(
                        xT_sb[:, k * NPCP : (k + 1) * NPCP],
                        rsld[:, k, :],
                        AF.Relu,
                        bias=b1_sb[:, k : k + 1],
                    )

                with (
                    tc.tile_pool(name="psH", bufs=3, space="PSUM") as psh_p,
                    tc.tile_pool(name="psS", bufs=2, space="PSUM") as pss_p,
                ):
                  for n in range(NT):
                    psh = psh_p.tile([128, H], dt.float32)
                    pss = pss_p.tile([128, 2], dt.float32)
                    for fo in range(0, H, 512):
                        for k in range(H_T):
                            nc.tensor.matmul(
                                psh[:, fo : fo + 512],
                                xT_sb[:, k * NPCP + n * 128 : k * NPCP + (n + 1) * 128],
                                w2_sb[:, k, fo : fo + 512],
                                start=(k == 0),
                                stop=(k == H_T - 1),
                            )
                    for k in range(H_T):
                        nc.tensor.matmul(
                            pss[:],
                            xT_sb[:, k * NPCP + n * 128 : k * NPCP + (n + 1) * 128],
                            w2_sb[:, k, H : H + 2],
                            start=(k == 0),
                            stop=(k == H_T - 1),
                        )
                    pay = paypool.tile([128, PAY], dt.uint8)
                    nc.vector.tensor_copy(
                        pay[:, 0:H].bitcast(dt.float8e4), psh[:]
                    )
                    side = pay[:, H:PAY].bitcast(dt.bfloat16)
                    nc.vector.tensor_copy(side[:, 0:1], pss[:, 0:1])
                    nc.vector.memset(side[:, 1:2], 1.0)
                    nc.vector.memset(side[:, 2:128], 0.0)
                    nc.vector.tensor_copy(er_sb[:, n : n + 1], pss[:, 1:2])
                    d = nc.sync.dma_start(
                        pay_local[n * 128 : (n + 1) * 128, :], pay[:]
                    )
                    pay_dmas.append(d)
                nc.vector.tensor_copy(er_bf[:], er_sb[:])

            # ---------------- AllGather payload ---------------------------
            if not do_ag:
                do_b = False
            cc = None
            if do_ag:
              cc = nc.gpsimd.collective_compute(
                "AllGather",
                ALU.bypass,
                replica_groups=[list(range(NC))],
                ins=[pay_local[:]],
                outs=[pay_full[:]],
              )
            if cc is not None:
              for d in pay_dmas:
                add_dep_helper(cc.ins, d.ins, sync=True, reason="cc after payload")

            # ---------------- phase B: edge aggregation -------------------
            if do_b:
              with (
                tc.tile_pool(name="phB", bufs=1) as bp,
                tc.tile_pool(name="erbc", bufs=2) as ebp,
                tc.tile_pool(name="gat", bufs=5) as gp,
                tc.tile_pool(name="lw", bufs=4) as lwp,
                tc.tile_pool(name="psAgg", bufs=1, space="PSUM") as psagg,
                tc.tile_pool(name="psEr", bufs=2, space="PSUM") as pser,
                tc.tile_pool(name="small", bufs=4) as smp,
            ):
                gidx_sb = bp.tile([128, EPC // 16], dt.int16)
                nc.sync.dma_start(gidx_sb[:], gidx[:])
                dl_sb = bp.tile([128, NBT], dt.bfloat16)
                nc.sync.dma_start(dl_sb[:], dstloc[:])

                for t in range(NT):
                    nbt = nblk_t[t]
                    # er_bc[e, d] = er[tile t][d]  — 2-matmul partition broadcast
                    erp1 = pser.tile([1, 128], dt.float32, tag="erp1")
                    nc.tensor.matmul(erp1[:], er_bf[:, t : t + 1], ident[:])
                    erow = smp.tile([1, 128], dt.bfloat16, tag="erow")
                    nc.vector.tensor_copy(erow[:], erp1[:])
                    erp2 = pser.tile([128, 128], dt.float32, tag="erp2")
                    nc.tensor.matmul(erp2[:], ones1_bf[:], erow[:])
                    er_bc = ebp.tile([128, 128], dt.bfloat16, tag="erbc")
                    nc.vector.tensor_copy(er_bc[:], erp2[:])

                    ps0 = psagg.tile([128, 512], dt.float32, tag="agg0")
                    ps1 = psagg.tile([128, 512], dt.float32, tag="agg1")
                    psz = psagg.tile([128, 1], dt.float32, tag="aggz")

                    for c in range(nbt // CB):
                        gt = gp.tile([128, CB, PAY], dt.uint8, tag="gat")
                        icol = (blk_base[t] + c * CB) * 8
                        gd = nc.gpsimd.dma_gather(
                            gt[:],
                            pay_full[:],
                            gidx_sb[:, icol : icol + CB * 8],
                            CB * 128,
                            CB * 128,
                            PAY,
                        )
                        add_dep_helper(gd.ins, lib_inst.ins, sync=False,
                                       reason="gather after lib")
                        add_dep_helper(gd.ins, cc.ins, sync=True,
                                       reason="gather after allgather")
                        blk0 = c * CB
                        # whole-chunk edge softmax weights (CB blocks at once,
                        # per-block values read through stride-0 broadcasts)
                        el_b = (
                            gt[:, :, H : H + 2]
                            .bitcast(dt.bfloat16)
                            .broadcast_to([128, CB, 128])
                        )
                        er_b = er_bc[:].rearrange(
                            "p (c d) -> p c d", c=1
                        ).broadcast_to([128, CB, 128])
                        es = lwp.tile([128, CB, 128], dt.bfloat16, tag="es")
                        nc.vector.tensor_tensor(es[:], er_b, el_b, op=ALU.add)
                        # lr = leaky_relu(es) = max(0.2*es, es)
                        lr = lwp.tile([128, CB, 128], dt.bfloat16, tag="lr")
                        nc.vector.scalar_tensor_tensor(
                            lr[:], es[:], 0.2, es[:], op0=ALU.mult, op1=ALU.max
                        )
                        # w = exp(lr)
                        wt = lwp.tile([128, CB, 128], dt.bfloat16, tag="wt")
                        nc.scalar.activation(wt[:], lr[:], AF.Exp)
                        # lw = (iota == dstloc) * w
                        iota_b = iota_bf[:].rearrange(
                            "p (c d) -> p c d", c=1
                        ).broadcast_to([128, CB, 128])
                        dl_b = (
                            dl_sb[:, blk_base[t] + blk0 : blk_base[t] + blk0 + CB]
                            .rearrange("p (c o) -> p c o", o=1)
                            .broadcast_to([128, CB, 128])
                        )
                        eq = lwp.tile([128, CB, 128], dt.bfloat16, tag="eq")
                        nc.vector.tensor_tensor(eq[:], iota_b, dl_b, op=ALU.is_equal)
                        lw = lwp.tile([128, CB, 128], dt.bfloat16, tag="lw")
                        nc.vector.tensor_tensor(lw[:], eq[:], wt[:], op=ALU.mult)
                        for b in range(CB):
                            blk = blk0 + b
                            first = blk == 0
                            last = blk == nbt - 1
                            h8 = gt[:, b, 0:H].bitcast(dt.float8e4)
                            one_b = gt[:, b, H + 2 : H + 4].bitcast(dt.bfloat16)
                            nc.tensor.matmul(
                                ps0[:], lw[:, b, :], h8[:, 0:512],
                                start=first, stop=last,
                            )
                            nc.tensor.matmul(
                                ps1[:], lw[:, b, :], h8[:, 512:1024],
                                start=first, stop=last,
                            )
                            nc.tensor.matmul(
                                psz[:], lw[:, b, :], one_b[:],
                                start=first, stop=last,
                            )

                    zc = smp.tile([128, 1], dt.float32, tag="zc")
                    nc.vector.tensor_scalar_max(zc[:], psz[:], 1e-30)
                    rz = smp.tile([128, 1], dt.float32, tag="rz")
                    nc.vector.reciprocal(rz[:], zc[:])
                    nc.scalar.mul(xg_sb[:, t * H : t * H + 512], ps0[:], rz[:])
                    nc.scalar.mul(xg_sb[:, t * H + 512 : (t + 1) * H], ps1[:], rz[:])

            # ---------------- transpose xg + AllGather xgT ----------------
            cc2 = None
            if do_b:
                with tc.tile_pool(name="phT", bufs=1) as tp:
                    xgT_sb = tp.tile([128, H_T, NPCP], dt.bfloat16)
                    with tc.tile_pool(name="psT", bufs=4, space="PSUM") as pst_p:
                        for t in range(NT):
                            for k in range(H_T):
                                pst = pst_p.tile([128, 128], dt.bfloat16, tag="pst")
                                nc.tensor.transpose(
                                    pst[:],
                                    xg_sb[:, t * H + k * 128 : t * H + (k + 1) * 128],
                                    ident[:],
                                )
                                nc.vector.tensor_copy(
                                    xgT_sb[:, k, t * 128 : (t + 1) * 128],
                                    pst[:],
                                )
                    xd = nc.sync.dma_start(
                        xgT_local.ap().rearrange("(k p) n -> p k n", p=128),
                        xgT_sb[:],
                    )
                cc2 = nc.gpsimd.collective_compute(
                    "AllGather",
                    ALU.bypass,
                    replica_groups=[list(range(NC))],
                    ins=[xgT_local[:]],
                    outs=[xgT_full[:]],
                )
                add_dep_helper(cc2.ins, xd.ins, sync=True, reason="cc2 after xgT")

            # ---------------- phase C: logits ----------------------------
            if not do_c:
                dum = paypool.tile([128, 512], dt.uint8, tag="dum")
                nc.vector.memset(dum[:], 0)
                nc.sync.dma_start(out[0:128, 0:512], dum[:])
            if do_c:
              GB = [(0, 512), (512, 512), (1024, 257)]  # (col offset, psum width)
              with (
                tc.tile_pool(name="phC", bufs=1) as cpc,
                tc.tile_pool(name="xtp", bufs=3) as xtp,
                tc.tile_pool(name="outp", bufs=6) as outp,
                tc.tile_pool(name="psC", bufs=2, space="PSUM") as psc_p,
              ):
                go_sb = cpc.tile([128, H_T, GOC], dt.bfloat16)
                nc.sync.dma_start(
                    go_sb[:], goc.ap().rearrange("(k p) g -> p k g", p=128)
                )
                rad_sb = cpc.tile([1, GPC], dt.float32)
                nc.sync.dma_start(rad_sb[:], radc[:])
                rad_bc = cpc.tile([128, GPC], dt.bfloat16)
                for g0, w in GB:
                    wr = min(w, GPC - g0)
                    psr = psc_p.tile([128, wr], dt.float32, tag="psrad")
                    nc.tensor.matmul(psr[:], ones1[:], rad_sb[:, g0 : g0 + wr])
                    nc.vector.tensor_copy(rad_bc[:, g0 : g0 + wr], psr[:])

                for m in range(NC * NT):
                    bk, tl = divmod(m, NT)
                    row0 = bk * NPC + tl * 128
                    nrows = min(128, NPC - tl * 128)
                    xt = xtp.tile([128, H_T, 128], dt.bfloat16, tag="xt")
                    xld = nc.sync.dma_start(
                        xt[:],
                        xgT_full.ap()[
                            bk * H : (bk + 1) * H, tl * 128 : (tl + 1) * 128
                        ].rearrange("(k p) n -> p k n", p=128),
                    )
                    add_dep_helper(xld.ins, cc2.ins, sync=True,
                                   reason="xt after xgT allgather")
                    pss = [
                        psc_p.tile(
                            [128, w], dt.float32, tag=f"psc{i}", name=f"psc{i}"
                        )
                        for i, (g0, w) in enumerate(GB)
                    ]
                    for k in range(H_T):
                        for i, (g0, w) in enumerate(GB):
                            nc.tensor.matmul(
                                pss[i][:],
                                xt[:, k, :],
                                go_sb[:, k, g0 : g0 + w],
                                start=(k == 0),
                                stop=(k == H_T - 1),
                            )
                    for i, (g0, w) in enumerate(GB):
                        wo = min(w, GPC - g0)
                        st = outp.tile([128, wo], dt.bfloat16, tag=f"st{i}")
                        nc.vector.scalar_tensor_tensor(
                            st[:],
                            pss[i][:, 0:wo],
                            pss[2][:, 256:257],
                            rad_bc[:, g0 : g0 + wo],
                            op0=ALU.add,
                            op1=ALU.add,
                        )
                        ot = outp.tile([128, wo], dt.float32, tag=f"ot{i}")
                        nc.scalar.activation(ot[:], st[:], AF.Sigmoid)
                        # quantize to uint8: trunc(255*sigmoid + 0.5)
                        ou = outp.tile([128, wo], dt.uint8, tag=f"ou{i}")
                        nc.vector.tensor_scalar(
                            ou[:], ot[:], 255.0, 0.5, op0=ALU.mult, op1=ALU.add
                        )
                        nc.sync.dma_start(
                            out[row0 : row0 + nrows, g0 : g0 + wo],
                            ou[0:nrows, :],
                        )

    nc.compile()
    return nc


# ---------------------------------------------------------------- entry point
def _assemble(results):
    full = np.empty((N, G), np.float32)
    for c in range(NC):
        g0 = c * GPC
        gw = min(GPC, G - g0)
        r = np.asarray(results[c]["out"])  # [N, GPC] uint8, rows in global order
        np.multiply(
            r[:, :gw],
            np.float32(1.0 / 255.0),
            out=full[:, g0 : g0 + gw],
        )
    return full


def kernel(**inputs):
    from concourse.bass_utils import run_bass_kernel_spmd

    in_maps, nblk_t = _host_prep(inputs)
    nc = build_nc(nblk_t)
    res = run_bass_kernel_spmd(nc, in_maps, list(range(NC)))
    return _assemble(res.results)


if __name__ == "__main__":
    # quick self-run with random data (no reference check)
    rng = np.random.default_rng(0)
    ins = {
        "features": rng.standard_normal((N, IN)).astype(np.float32),
        "src": rng.integers(0, N, E),
        "dst": rng.integers(0, N, E),
        "W1": rng.standard_normal((IN, H)).astype(np.float32) * 0.02,
        "b1": np.zeros(H, np.float32),
        "fc_w": rng.standard_normal((H, H)).astype(np.float32) * 0.02,
        "attn_l": rng.standard_normal(H).astype(np.float32) * 0.02,
        "attn_r": rng.standard_normal(H).astype(np.float32) * 0.02,
        "gat_bias": np.zeros(H, np.float32),
        "go_embed": rng.standard_normal((G + NZ, H)).astype(np.float32) * 0.02,
        "go_rad": rng.standard_normal((G + NZ, 1)).astype(np.float32) * 0.02,
        "rel_embed": rng.standard_normal((R + 1, H)).astype(np.float32) * 0.02,
    }
    out = kernel(**ins)
    print("out", out.shape, out.dtype, out[:2, :4])
